# revision 24
# baseline (speedup 1.0000x reference)
"""DGL-JTNN encoder forest message passing on 8 Trainium2 NeuronCores.

Strategy: data-parallel over trees (16 complete binary trees per core, depth 6).
The forest built by the reference's ``_build_forest`` is deterministic complete
binary trees in BFS order, so the per-level segment-sums collapse into dense
ops.  v2 layout: within each level, nodes are DE-INTERLEAVED (all left children
first, then all right children, recursively), so

  * bottom-up pair-sum:  U = m[left half] + m[right half]       (contiguous TT)
  * top-down rep2:       s = T[parent slice] - m_up[half]        (contiguous TT)
  * rep2 injects become outer-repeat APs (one matmul, plain slices)

which keeps every DVE op in its fast 2x (bf16, step-1) mode.

All x-projections are folded into the embedding on the host (E@Wz1, E@Wh1,
E@Wr, E@Wg1 over the 780-entry vocab, then gathered per node); additionally the
whole leaf-level message m_leaf = sigmoid(az)*tanh(ah) is a per-vocab table
(leaves receive no messages), so it is gathered host-side and DMA'd straight
into the edge-state tile.  az/ah therefore only cover non-leaf nodes.

Feature-major tensors are [128, 4, N] SBUF tiles (feature 450 split
128/128/128/66); matmuls run bf16 with fp32 PSUM, PSUM tags rotated
round-robin; each PSUM bank carries exactly one start/stop so
recurrence-independent injects issue before recurrence-dependent matmuls.
Tiny levels use ar/ag injects (4 matmuls) instead of direct Wr/Wg1 passes
(16 matmuls) to cut the LDWEIGHTS-bound instruction count in the
latency-bound middle cascade.

DMA: inputs are spread need-ordered across the three DMA rings (sync HWDGE,
scalar HWDGE, gpsimd SWDGE); outputs (bf16) go on the sync ring.
"""

import sys

for _p in ("/opt/trn_rl_repo", "/root/.axon_site/_ro/trn_rl_repo"):
    if _p not in sys.path:
        sys.path.append(_p)

from contextlib import ExitStack

import numpy as np
import ml_dtypes

import concourse.bass as bass
import concourse.tile as tile
from concourse import bacc
from concourse import mybir
from concourse.bass_utils import run_bass_kernel_spmd
from concourse.masks import make_identity

F32 = mybir.dt.float32
BF16 = mybir.dt.bfloat16
SIG = mybir.ActivationFunctionType.Sigmoid
TANH = mybir.ActivationFunctionType.Tanh
RELU = mybir.ActivationFunctionType.Relu
ADD = mybir.AluOpType.add
SUB = mybir.AluOpType.subtract
MUL = mybir.AluOpType.mult

BF16NP = ml_dtypes.bfloat16

B, DEPTH, NPT, H, V = 128, 6, 127, 450, 780
NCORES = 8
TPC = B // NCORES                     # 16 trees per core
LVL_N = [TPC * (1 << l) for l in range(DEPTH + 1)]      # 16..1024
LVL_OFF = [0]
for n in LVL_N:
    LVL_OFF.append(LVL_OFF[-1] + n)
NN = LVL_OFF[-1]                      # 2032 nodes per core
NE = NN - TPC                         # 2016 up-edges per core
NL = LVL_OFF[DEPTH]                   # 1008 non-leaf cols
O6 = LVL_OFF[DEPTH]                   # leaf level node offset (1008)
E6 = O6 - TPC                         # leaf edge offset (992)
KT = [128, 128, 128, 66]              # feature K-chunk sizes (450 total)
KO = [0, 128, 256, 384]
CH = 256                              # N-chunk per pipeline step

# device weight blocks: (key, source tensor name, row offset)
WKEYS = [("wz2", "Wz", H), ("wh2", "Wh", H), ("ur", "Ur", 0), ("wg2", "Wg", H)]

_CACHE = {}


def _build_program():
    nc = bacc.Bacc("TRN2", target_bir_lowering=False, debug=False)

    az_d = nc.dram_tensor("azi", [128, 4, NL], BF16, kind="ExternalInput").ap()
    ah_d = nc.dram_tensor("ahi", [128, 4, NL], BF16, kind="ExternalInput").ap()
    ag_d = nc.dram_tensor("agi", [128, 4, NN], BF16, kind="ExternalInput").ap()
    # ar and the leaf-m table are split so every startup DMA is a whole
    # contiguous tensor (2KB+ per-partition lines; strided slices run ~3x
    # slower on the SDMA path)
    ar5_d = nc.dram_tensor("ar5i", [128, 4, LVL_N[5]], BF16, kind="ExternalInput").ap()
    ar04_d = nc.dram_tensor("ar04i", [128, 4, LVL_OFF[5]], BF16,
                            kind="ExternalInput").ap()
    mla_d = nc.dram_tensor("mlai", [128, 4, 512], BF16, kind="ExternalInput").ap()
    mlb_d = nc.dram_tensor("mlbi", [128, 4, 512], BF16, kind="ExternalInput").ap()
    scr_d = nc.dram_tensor("scr", [128, 8], BF16, kind="ExternalOutput").ap()
    w_dram = {key: nc.dram_tensor(key, [128, 4, 512], BF16, kind="ExternalInput").ap()
              for key, _, _ in WKEYS}
    out_d = nc.dram_tensor("hT", [H, NN], BF16, kind="ExternalOutput").ap()

    with tile.TileContext(nc) as tc, ExitStack() as ctx:
        pers = ctx.enter_context(tc.tile_pool(name="pers", bufs=1))
        work = ctx.enter_context(tc.tile_pool(name="work", bufs=2))
        dmp = ctx.enter_context(tc.tile_pool(name="dmp", bufs=2))
        ps = ctx.enter_context(tc.tile_pool(name="ps", bufs=1, space="PSUM"))

        # ---- persistent projection/state tiles ----
        az = pers.tile([128, 4, NL], BF16, name="az", tag="az")
        ah = pers.tile([128, 4, NL], BF16, name="ah", tag="ah")
        ag = pers.tile([128, 4, NN], BF16, name="ag", tag="ag")
        ar5 = pers.tile([128, 4, LVL_N[5]], BF16, name="ar5", tag="ar5")
        ar04 = pers.tile([128, 4, LVL_OFF[5]], BF16, name="ar04", tag="ar04")
        mup = pers.tile([128, 4, NE - LVL_N[DEPTH]], BF16, name="mup", tag="mup")
        mup6a = pers.tile([128, 4, 512], BF16, name="mup6a", tag="mup6a")
        mup6b = pers.tile([128, 4, 512], BF16, name="mup6b", tag="mup6b")
        rmup = pers.tile([128, 4, NE], BF16, name="rmup", tag="rmup")
        U = pers.tile([128, 4, NL], BF16, name="U", tag="U")
        Urm = pers.tile([128, 4, NL], BF16, name="Urm", tag="Urm")

        def ar_ap(o):
            # ar columns [0, 496) live in ar04, [496, 1008) in ar5
            return (ar5, o - LVL_OFF[5]) if o >= LVL_OFF[5] else (ar04, o)

        def ml_ap(j):
            # leaf-edge m columns [0, 512) live in mup6a, [512, 1024) in mup6b
            return (mup6a, j) if j < 512 else (mup6b, j - 512)
        wb = {key: pers.tile([128, 4, 512], BF16, name=f"w_{key}", tag=f"w_{key}")
              for key, _, _ in WKEYS}
        ident_bf = pers.tile([128, 128], BF16, name="ident_bf", tag="ident_bf")

        ps_tags = ["pz", "ph", "pr", "pg"]
        rot = [0]

        # identity generated on-device (no DMA on the critical path)
        make_identity(nc, ident_bf[:])

        # ---- startup DMAs.  Whole tensors (2KB+ per-partition lines run at
        # ~300+ GB/s; strided slices run ~3x slower).  Wave 1 carries only
        # the leaf-critical bytes (ar5 / ur / ml) on the two HWDGE rings; the
        # bulk tensors sit behind gate DMAs reading compute-produced rmup so
        # they can't steal wave-1 HBM bandwidth.
        nc.sync.dma_start(ar5[:], ar5_d[:])
        nc.sync.dma_start(wb["wz2"][:], w_dram["wz2"][:])
        nc.sync.dma_start(wb["wh2"][:], w_dram["wh2"][:])
        nc.scalar.dma_start(wb["ur"][:], w_dram["ur"][:])
        nc.scalar.dma_start(mup6a[:], mla_d[:])
        nc.scalar.dma_start(mup6b[:], mlb_d[:])
        # gpsimd ring: gate on leaf chunk-0 compute, then the bulk loads
        nc.gpsimd.dma_start(scr_d[:, 2:3], rmup[:, 0, E6:E6 + 1])
        nc.gpsimd.dma_start(ah[:], ah_d[:])
        nc.gpsimd.dma_start(wb["wg2"][:], w_dram["wg2"][:])

        def junk_mms(n, nn=128):
            # Real matmul burst into a scratch PSUM tile: the HAM clock-gate
            # only counts MATMUL activity (LDWEIGHTS doesn't un-throttle it),
            # and un-throttling needs a ~3.4us contiguous busy stretch.  The
            # results are never read.
            jp = ps.tile([128, 4, CH], F32, name="jp", tag=ps_tags[rot[0] % 4])
            rot[0] += 1
            rhs = ident_bf[:, :nn] if nn <= 128 else wb["ur"][:, 0, :nn]
            for i in range(n):
                nc.tensor.matmul(out=jp[:, 0, :nn], lhsT=ident_bf[:],
                                 rhs=rhs, start=(i == 0), stop=(i == n - 1))

        junk_mms(44)   # DMA-wait warm-up: ~4.7us of cold-clock MM activity

        def stage_b():
            # after leaf chunk 0: ar04/az behind a gate on the scalar ring;
            # ag queues on sync naturally behind the wave-1 weights
            nc.scalar.dma_start(scr_d[:, 0:1], rmup[:, 0, E6:E6 + 1])
            nc.scalar.dma_start(ar04[:], ar04_d[:])
            nc.scalar.dma_start(az[:], az_d[:])
            for a, b in [(0, NL), (NL, NN)]:
                nc.sync.dma_start(ag[:, :, a:b], ag_d[:, :, a:b])

        def ps_tile():
            t = ps.tile([128, 4, CH], F32, name="pp", tag=ps_tags[rot[0] % 4])
            rot[0] += 1
            return t

        def act2(out, in_, func):
            # split activation into two M-tile halves so downstream consumers
            # (per-k matmuls, DVE) start after half the work
            nc.scalar.activation(out[:, :2], in_[:, :2], func)
            nc.scalar.activation(out[:, 2:], in_[:, 2:], func)

        def tt2(eng, out, in0, in1, op):
            eng.tensor_tensor(out=out[:, :2], in0=in0[:, :2], in1=in1[:, :2], op=op)
            eng.tensor_tensor(out=out[:, 2:], in0=in0[:, 2:], in1=in1[:, 2:], op=op)

        def mm_pass(pt, nn, terms=(), inject=None, first=False, last=False):
            """Emit one ordered batch of matmuls accumulating into pt[:, :, :nn].

            PSUM ``has_written`` is per-element but ``start=True`` clears the
            whole 2KB bank, so a tile's matmuls carry exactly one start (first
            MM per bank, on the ``first=True`` batch) and one stop (last MM
            per bank, ``last=True`` batch).  terms: (weight_tile, rhs_fn(k))
            with [K, nn] APs.  inject: rhs_fn(m) returning a [128, nn] slice
            or a [128, 2, nn/2] outer-repeat AP, added via one identity-matmul
            per M-tile.
            """
            seq = []
            if inject is not None:
                for m in range(4):
                    seq.append((m, ident_bf[:], inject(m)))
            for wt, rhs_fn in terms:
                for m in range(4):
                    for k in range(4):
                        seq.append((m, wt[:KT[k], k, 128 * m:128 * (m + 1)], rhs_fn(k)))
            fb, lb = {}, {}
            for i, (m, _, _) in enumerate(seq):
                fb.setdefault(m // 2, i)
                lb[m // 2] = i
            for i, (m, lhsT, rhs) in enumerate(seq):
                out = pt[:, m, :nn]
                if len(rhs.shape) == 3:
                    out = out.rearrange("p (a b) -> p a b", a=2)
                nc.tensor.matmul(out=out, lhsT=lhsT, rhs=rhs,
                                 start=(first and fb[m // 2] == i),
                                 stop=(last and lb[m // 2] == i))

        def inj(t, o, n):         # plain inject of projection t cols [o, o+n)
            return lambda m: t[:, m, o:o + n]

        def inj_par(t, po, P, n0, nn):
            # inject indexed by parent: col j -> parent po + (j mod P)
            if nn <= P:
                pp = po + (n0 % P)
                return lambda m: t[:, m, pp:pp + nn]
            # chunk spans both halves (nn == 2P): outer-repeat the parent slice
            return lambda m: t[:, m:m + 1, po:po + P].broadcast_to((128, 2, P))

        # ============ leaf level (bottom-up l=6) ============
        # m_leaf comes pre-computed via DMA; device does the r-gate only.
        po6, P6 = LVL_OFF[DEPTH - 1], LVL_N[DEPTH] // 2
        for c in range(LVL_N[DEPTH] // CH):
            n0 = c * CH
            mlt, mo = ml_ap(n0)
            ms = mlt[:, :, mo:mo + CH]
            rms = rmup[:, :, E6 + n0:E6 + n0 + CH]

            pr = ps_tile()
            pp = n0 % P6
            mm_pass(pr, CH, inject=inj(ar5, pp, CH), first=True)
            mm_pass(pr, CH, [(wb["ur"], lambda k: mlt[:KT[k], k, mo:mo + CH])],
                    last=True)
            r = work.tile([128, 4, CH], BF16, name="r", tag="r")
            act2(r[:, :, :CH], pr[:, :, :CH], SIG)
            tt2(nc.vector, rms, r[:, :, :CH], ms, MUL)

            # pair-sums once the matching right-half chunk is done
            if c >= 2:
                j0 = (c - 2) * CH
                tt2(nc.gpsimd, U[:, :, po6 + j0:po6 + j0 + CH],
                    mup6a[:, :, j0:j0 + CH], mup6b[:, :, j0:j0 + CH], ADD)
                tt2(nc.vector, Urm[:, :, po6 + j0:po6 + j0 + CH],
                    rmup[:, :, E6 + j0:E6 + j0 + CH],
                    rmup[:, :, E6 + P6 + j0:E6 + P6 + j0 + CH], ADD)
            if c == 0:
                stage_b()

        # ================= phase 1: bottom-up (levels 5..1) =================
        for l in range(DEPTH - 1, 0, -1):
            L, o = LVL_N[l], LVL_OFF[l]
            e0, po, P = o - TPC, LVL_OFF[l - 1], LVL_N[l] // 2
            nch = max(1, L // CH)
            chn = min(CH, L)
            for c in range(nch):
                n0 = c * chn
                nn = chn
                ms = mup[:, :, e0 + n0:e0 + n0 + nn]
                rms = rmup[:, :, e0 + n0:e0 + n0 + nn]

                # recurrence-independent batches first
                pz = ps_tile()
                ph = ps_tile()
                pr = ps_tile()
                mm_pass(pz, nn, inject=inj(az, o + n0, nn), first=True)
                mm_pass(ph, nn, inject=inj(ah, o + n0, nn), first=True)
                mm_pass(pr, nn, inject=inj_par(ar04, po, P, n0, nn), first=True)

                z = work.tile([128, 4, CH], BF16, name="z", tag="z")
                mt = work.tile([128, 4, CH], BF16, name="mt", tag="mt")
                mm_pass(ph, nn, [(wb["wh2"], lambda k: Urm[:KT[k], k, o + n0:o + n0 + nn])],
                        last=True)
                act2(mt[:, :, :nn], ph[:, :, :nn], TANH)

                mm_pass(pz, nn, [(wb["wz2"], lambda k: U[:KT[k], k, o + n0:o + n0 + nn])],
                        last=True)
                act2(z[:, :, :nn], pz[:, :, :nn], SIG)

                s_ap = U[:, :, o + n0:o + n0 + nn]
                t1 = work.tile([128, 4, CH], BF16, name="t1", tag="t1")
                tt2(nc.vector, t1[:, :, :nn], mt[:, :, :nn], s_ap, SUB)
                t2 = work.tile([128, 4, CH], BF16, name="t2", tag="t2")
                tt2(nc.vector, t2[:, :, :nn], t1[:, :, :nn], z[:, :, :nn], MUL)
                tt2(nc.vector, ms, t2[:, :, :nn], s_ap, ADD)

                mm_pass(pr, nn, [(wb["ur"], lambda k: mup[:KT[k], k, e0 + n0:e0 + n0 + nn])],
                        last=True)
                r = work.tile([128, 4, CH], BF16, name="r", tag="r")
                act2(r[:, :, :nn], pr[:, :, :nn], SIG)
                tt2(nc.vector, rms, r[:, :, :nn], ms, MUL)

                # pair-sums to the parent level, chunk-wise as halves complete
                if c == nch - 1:
                    for j0 in range(0, P, CH):
                        pn = min(CH, P - j0)
                        tt2(nc.gpsimd, U[:, :, po + j0:po + j0 + pn],
                            mup[:, :, e0 + j0:e0 + j0 + pn],
                            mup[:, :, e0 + P + j0:e0 + P + j0 + pn], ADD)
                        tt2(nc.vector, Urm[:, :, po + j0:po + j0 + pn],
                            rmup[:, :, e0 + j0:e0 + j0 + pn],
                            rmup[:, :, e0 + P + j0:e0 + P + j0 + pn], ADD)
            if L <= 128:
                junk_mms(6)   # hold clock-gate density through the cascade

        # ================= roots output =================
        pg = ps_tile()
        mm_pass(pg, TPC, inject=inj(ag, 0, TPC), first=True)
        mm_pass(pg, TPC, [(wb["wg2"], lambda k: U[:KT[k], k, 0:TPC])], last=True)
        h0 = work.tile([128, 4, CH], BF16, name="h", tag="h")
        nc.scalar.activation(h0[:, :, :TPC], pg[:, :, :TPC], RELU)
        nc.sync.dma_start(out_d[0:384, 0:TPC].rearrange("(k p) c -> p k c", p=128),
                          h0[:, :3, :TPC])
        nc.sync.dma_start(out_d[384:450, 0:TPC], h0[:66, 3, :TPC])
        junk_mms(6)

        # ================= phase 2: top-down =================
        Tn = Trn = None
        for l in range(1, DEPTH + 1):
            L, o = LVL_N[l], LVL_OFF[l]
            e0, po, P = o - TPC, LVL_OFF[l - 1], LVL_N[l] // 2
            if l == 1:
                T_ap, Trm_ap = U[:, :, 0:TPC], Urm[:, :, 0:TPC]
            else:
                T_ap, Trm_ap = Tn[:, :, :P], Trn[:, :, :P]

            if l < DEPTH:
                Dm = dmp.tile([128, 4, LVL_N[DEPTH - 1]], BF16, name="Dm", tag="Dm")
                Drm = dmp.tile([128, 4, LVL_N[DEPTH - 1]], BF16, name="Drm", tag="Drm")
                Tn = dmp.tile([128, 4, 512], BF16, name="Tn", tag="Tn")
                Trn = dmp.tile([128, 4, 512], BF16, name="Trn", tag="Trn")

            nch = max(1, L // CH)
            chn = min(CH, L)
            for c in range(nch):
                n0 = c * chn
                nn = chn
                if l == DEPTH:
                    mlt, mo = ml_ap(n0)
                    mslice = mlt[:, :, mo:mo + nn]

                    def msl(a, b, mlt=mlt, mo=mo, n0=n0):
                        t, o2 = ml_ap(n0 + a)
                        return t[:, :, o2:o2 + b - a]
                else:
                    mslice = mup[:, :, e0 + n0:e0 + n0 + nn]

                    def msl(a, b, e0=e0, n0=n0):
                        return mup[:, :, e0 + n0 + a:e0 + n0 + b]
                rmslice = rmup[:, :, e0 + n0:e0 + n0 + nn]

                # recurrence-independent batches first
                pz = ps_tile()
                ph = ps_tile()
                pr = ps_tile() if l < DEPTH else None
                pg = ps_tile()
                mm_pass(pz, nn, inject=inj_par(az, po, P, n0, nn), first=True)
                mm_pass(ph, nn, inject=inj_par(ah, po, P, n0, nn), first=True)
                if pr is not None:
                    art, aro = ar_ap(o + n0)
                    mm_pass(pr, nn, inject=inj(art, aro, nn), first=True)
                mm_pass(pg, nn, inject=inj(ag, o + n0, nn), first=True)

                # s = T[parent] - m_up ; arm = Trm[parent] - rm_up
                # (contiguous thanks to the de-interleaved level layout)
                s = work.tile([128, 4, CH], BF16, name="s", tag="s")
                arm = work.tile([128, 4, CH], BF16, name="arm", tag="arm")
                if nn <= P:
                    pp = n0 % P
                    tt2(nc.vector, s[:, :, :nn], T_ap[:, :, pp:pp + nn], mslice, SUB)
                    tt2(nc.vector, arm[:, :, :nn], Trm_ap[:, :, pp:pp + nn],
                        rmslice, SUB)
                else:
                    tt2(nc.vector, s[:, :, :P], T_ap[:, :, 0:P], msl(0, P), SUB)
                    tt2(nc.vector, s[:, :, P:2 * P], T_ap[:, :, 0:P],
                        msl(P, 2 * P), SUB)
                    tt2(nc.vector, arm[:, :, :P], Trm_ap[:, :, 0:P],
                        rmup[:, :, e0 + n0:e0 + n0 + P], SUB)
                    tt2(nc.vector, arm[:, :, P:2 * P], Trm_ap[:, :, 0:P],
                        rmup[:, :, e0 + n0 + P:e0 + n0 + 2 * P], SUB)

                mm_pass(ph, nn, [(wb["wh2"], lambda k: arm[:KT[k], k, :nn])], last=True)
                mt = work.tile([128, 4, CH], BF16, name="mt", tag="mt")
                act2(mt[:, :, :nn], ph[:, :, :nn], TANH)

                mm_pass(pz, nn, [(wb["wz2"], lambda k: s[:KT[k], k, :nn])], last=True)
                z = work.tile([128, 4, CH], BF16, name="z", tag="z")
                act2(z[:, :, :nn], pz[:, :, :nn], SIG)

                if l < DEPTH:
                    dslice = Dm[:, :, n0:n0 + nn]
                else:
                    mb6 = work.tile([128, 4, CH], BF16, name="mb6", tag="nm")
                    dslice = mb6[:, :, :nn]
                t1 = work.tile([128, 4, CH], BF16, name="t1", tag="t1")
                tt2(nc.vector, t1[:, :, :nn], mt[:, :, :nn], s[:, :, :nn], SUB)
                t2 = work.tile([128, 4, CH], BF16, name="t2", tag="t2")
                tt2(nc.vector, t2[:, :, :nn], t1[:, :, :nn], z[:, :, :nn], MUL)
                tt2(nc.vector, dslice, t2[:, :, :nn], s[:, :, :nn], ADD)

                if l < DEPTH:
                    # r/rm feed the next level's arm; the last level has none
                    mm_pass(pr, nn, [(wb["ur"], lambda k: dslice[:KT[k], k, :])], last=True)
                    r = work.tile([128, 4, CH], BF16, name="r", tag="r")
                    act2(r[:, :, :nn], pr[:, :, :nn], SIG)
                    tt2(nc.vector, Drm[:, :, n0:n0 + nn], r[:, :, :nn], dslice, MUL)
                    # next level's Trm chunk
                    tt2(nc.vector, Trn[:, :, n0:n0 + nn],
                        Urm[:, :, o + n0:o + n0 + nn], Drm[:, :, n0:n0 + nn], ADD)

                # fused final output; node_m doubles as next level's T chunk
                if l == DEPTH:
                    nm_fn = lambda k: dslice[:KT[k], k, :]
                else:
                    tt2(nc.gpsimd, Tn[:, :, n0:n0 + nn],
                        U[:, :, o + n0:o + n0 + nn], dslice, ADD)
                    nm_fn = lambda k: Tn[:KT[k], k, n0:n0 + nn]
                mm_pass(pg, nn, [(wb["wg2"], nm_fn)], last=True)
                h = work.tile([128, 4, CH], BF16, name="h", tag="h")
                nc.scalar.activation(h[:, :, :nn], pg[:, :, :nn], RELU)
                nc.sync.dma_start(
                    out_d[0:384, o + n0:o + n0 + nn].rearrange("(k p) c -> p k c", p=128),
                    h[:, :3, :nn])
                nc.sync.dma_start(out_d[384:450, o + n0:o + n0 + nn], h[:66, 3, :nn])
            if L <= 128:
                junk_mms(6)   # hold clock-gate density through the cascade
            if l == 3:
                # escape burst: one contiguous ~3.4us cold-clock MM stretch
                # re-arms the HAM SHORT window before the dense big levels
                junk_mms(16, nn=CH)

    nc.compile()
    return nc


def _perm_for_core(c):
    """Node permutation: level-major, de-interleaved within each level.

    order(0) = [(t, 0) for t in trees]; order(l) = lefts(order(l-1)) then
    rights(order(l-1)), so children of the parent at in-level position j sit
    at positions j (left) and j + P (right).
    """
    perm = []
    order = [(t, 0) for t in range(TPC)]
    for l in range(DEPTH + 1):
        if l > 0:
            order = [(t, 2 * i) for (t, i) in order] + \
                    [(t, 2 * i + 1) for (t, i) in order]
        base_l = (1 << l) - 1
        for t, i in order:
            perm.append((TPC * c + t) * NPT + base_l + i)
    return np.asarray(perm, dtype=np.int64)


def _pack_kfmt(mat, ncols=None):
    """[N, 450] fp32 -> [128, 4, ncols] bf16 K-chunk layout (transposed)."""
    n = mat.shape[0] if ncols is None else ncols
    out = np.zeros((128, 4, n), dtype=BF16NP)
    for k in range(4):
        out[:KT[k], k, :] = mat[:n, KO[k]:KO[k] + KT[k]].T.astype(BF16NP)
    return out


def _pack_weight(W, ro):
    """W[ro:ro+450, :450] fp32 -> [128, 4, 512] bf16 lhsT (M zero-padded)."""
    out = np.zeros((128, 4, 512), dtype=BF16NP)
    for k in range(4):
        out[:KT[k], k, :H] = W[ro + KO[k]:ro + KO[k] + KT[k], :].astype(BF16NP)
    return out


def kernel(**inputs):
    wid = np.ascontiguousarray(np.asarray(inputs["wid"], dtype=np.int32))
    emb = np.ascontiguousarray(np.asarray(inputs["emb"], dtype=np.float32))
    ws = {nm: np.ascontiguousarray(np.asarray(inputs[nm], dtype=np.float32))
          for nm in ("Wz", "Wh", "Wr", "Ur", "Wg")}
    # biases are zero-filled by the reference generator; folding nonzero ones
    # into the per-vocab projections would be needed otherwise.
    for bn in ("bz", "bh", "bur", "bg"):
        bv = np.asarray(inputs[bn])
        assert not np.any(bv), f"nonzero bias {bn} unsupported by this kernel"

    if "nc" not in _CACHE:
        _CACHE["nc"] = _build_program()
        _CACHE["perms"] = [_perm_for_core(c) for c in range(NCORES)]
    nc = _CACHE["nc"]
    perms = _CACHE["perms"]

    # fold the embedding into the per-vocab projections once per vocab entry
    EZ = emb @ ws["Wz"][:H]
    EH = emb @ ws["Wh"][:H]
    ER = emb @ ws["Wr"]
    EG = emb @ ws["Wg"][:H]
    # leaf-level message is a pure per-vocab function: m = sigmoid(az)*tanh(ah)
    ML = (1.0 / (1.0 + np.exp(-EZ.astype(BF16NP).astype(np.float32)))
          * np.tanh(EH.astype(BF16NP).astype(np.float32)))
    wmaps = {key: _pack_weight(ws[srcnm], ro) for key, srcnm, ro in WKEYS}
    in_maps = []
    for c in range(NCORES):
        w = wid[perms[c]]
        wnl, wlf = w[:NL], w[NL:]
        ERp = ER[wnl]
        MLp = ML[wlf]
        m = {"azi": _pack_kfmt(EZ[wnl]), "ahi": _pack_kfmt(EH[wnl]),
             "agi": _pack_kfmt(EG[w]),
             "ar04i": _pack_kfmt(ERp[:LVL_OFF[5]]),
             "ar5i": _pack_kfmt(ERp[LVL_OFF[5]:]),
             "mlai": _pack_kfmt(MLp[:512]), "mlbi": _pack_kfmt(MLp[512:])}
        m.update(wmaps)
        in_maps.append(m)

    res = run_bass_kernel_spmd(nc, in_maps, core_ids=list(range(NCORES)))
    _CACHE["last_result"] = res

    out = np.empty((B * NPT, H), dtype=np.float32)
    for c in range(NCORES):
        out[perms[c]] = res.results[c]["hT"].T.astype(np.float32)
    return out


# revision 28
# speedup vs baseline: 1.0689x; 1.0689x over previous
"""DGL-JTNN encoder forest message passing on 8 Trainium2 NeuronCores.

Strategy: data-parallel over trees (16 complete binary trees per core, depth 6).
The forest built by the reference's ``_build_forest`` is deterministic complete
binary trees in BFS order, so the per-level segment-sums collapse into dense
ops.  v2 layout: within each level, nodes are DE-INTERLEAVED (all left children
first, then all right children, recursively), so

  * bottom-up pair-sum:  U = m[left half] + m[right half]       (contiguous TT)
  * top-down rep2:       s = T[parent slice] - m_up[half]        (contiguous TT)
  * rep2 injects become outer-repeat APs (one matmul, plain slices)

which keeps every DVE op in its fast 2x (bf16, step-1) mode.

All x-projections are folded into the embedding on the host (E@Wz1, E@Wh1,
E@Wr, E@Wg1 over the 780-entry vocab, then gathered per node); additionally the
whole leaf-level message m_leaf = sigmoid(az)*tanh(ah) is a per-vocab table
(leaves receive no messages), so it is gathered host-side and DMA'd straight
into the edge-state tile.  az/ah therefore only cover non-leaf nodes.

Feature-major tensors are [128, 4, N] SBUF tiles (feature 450 split
128/128/128/66); matmuls run bf16 with fp32 PSUM, PSUM tags rotated
round-robin; each PSUM bank carries exactly one start/stop so
recurrence-independent injects issue before recurrence-dependent matmuls.
Tiny levels use ar/ag injects (4 matmuls) instead of direct Wr/Wg1 passes
(16 matmuls) to cut the LDWEIGHTS-bound instruction count in the
latency-bound middle cascade.

DMA: inputs are spread need-ordered across the three DMA rings (sync HWDGE,
scalar HWDGE, gpsimd SWDGE); outputs (bf16) go on the sync ring.
"""

import sys

for _p in ("/opt/trn_rl_repo", "/root/.axon_site/_ro/trn_rl_repo"):
    if _p not in sys.path:
        sys.path.append(_p)

from contextlib import ExitStack

import numpy as np
import ml_dtypes

import concourse.bass as bass
import concourse.tile as tile
from concourse import bacc
from concourse import mybir
from concourse.bass_utils import run_bass_kernel_spmd
from concourse.masks import make_identity

F32 = mybir.dt.float32
BF16 = mybir.dt.bfloat16
SIG = mybir.ActivationFunctionType.Sigmoid
TANH = mybir.ActivationFunctionType.Tanh
RELU = mybir.ActivationFunctionType.Relu
ADD = mybir.AluOpType.add
SUB = mybir.AluOpType.subtract
MUL = mybir.AluOpType.mult

BF16NP = ml_dtypes.bfloat16

B, DEPTH, NPT, H, V = 128, 6, 127, 450, 780
NCORES = 8
TPC = B // NCORES                     # 16 trees per core
LVL_N = [TPC * (1 << l) for l in range(DEPTH + 1)]      # 16..1024
LVL_OFF = [0]
for n in LVL_N:
    LVL_OFF.append(LVL_OFF[-1] + n)
NN = LVL_OFF[-1]                      # 2032 nodes per core
NE = NN - TPC                         # 2016 up-edges per core
NL = LVL_OFF[DEPTH]                   # 1008 non-leaf cols
O6 = LVL_OFF[DEPTH]                   # leaf level node offset (1008)
E6 = O6 - TPC                         # leaf edge offset (992)
KT = [128, 128, 128, 66]              # feature K-chunk sizes (450 total)
KO = [0, 128, 256, 384]
CH = 256                              # N-chunk per pipeline step

# device weight blocks: (key, source tensor name, row offset)
WKEYS = [("wz2", "Wz", H), ("wh2", "Wh", H), ("ur", "Ur", 0), ("wg2", "Wg", H)]

_CACHE = {}


def _build_program():
    nc = bacc.Bacc("TRN2", target_bir_lowering=False, debug=False)

    az_d = nc.dram_tensor("azi", [128, 4, NL], BF16, kind="ExternalInput").ap()
    ah_d = nc.dram_tensor("ahi", [128, 4, NL], BF16, kind="ExternalInput").ap()
    ag_d = nc.dram_tensor("agi", [128, 4, NN], BF16, kind="ExternalInput").ap()
    # ar and the leaf-m table are split so every startup DMA is a whole
    # contiguous tensor (2KB+ per-partition lines; strided slices run ~3x
    # slower on the SDMA path)
    ar5_d = nc.dram_tensor("ar5i", [128, 4, LVL_N[5]], BF16, kind="ExternalInput").ap()
    ar04_d = nc.dram_tensor("ar04i", [128, 4, LVL_OFF[5]], BF16,
                            kind="ExternalInput").ap()
    mla_d = nc.dram_tensor("mlai", [128, 4, 512], BF16, kind="ExternalInput").ap()
    mlb_d = nc.dram_tensor("mlbi", [128, 4, 512], BF16, kind="ExternalInput").ap()
    scr_d = nc.dram_tensor("scr", [128, 8], BF16, kind="ExternalOutput").ap()
    w_dram = {key: nc.dram_tensor(key, [128, 4, 512], BF16, kind="ExternalInput").ap()
              for key, _, _ in WKEYS}
    out_d = nc.dram_tensor("hT", [H, NN], BF16, kind="ExternalOutput").ap()

    with tile.TileContext(nc) as tc, ExitStack() as ctx:
        pers = ctx.enter_context(tc.tile_pool(name="pers", bufs=1))
        work = ctx.enter_context(tc.tile_pool(name="work", bufs=2))
        dmp = ctx.enter_context(tc.tile_pool(name="dmp", bufs=2))
        ps = ctx.enter_context(tc.tile_pool(name="ps", bufs=1, space="PSUM"))

        # ---- persistent projection/state tiles ----
        az = pers.tile([128, 4, NL], BF16, name="az", tag="az")
        ah = pers.tile([128, 4, NL], BF16, name="ah", tag="ah")
        ag = pers.tile([128, 4, NN], BF16, name="ag", tag="ag")
        ar5 = pers.tile([128, 4, LVL_N[5]], BF16, name="ar5", tag="ar5")
        ar04 = pers.tile([128, 4, LVL_OFF[5]], BF16, name="ar04", tag="ar04")
        mup = pers.tile([128, 4, NE - LVL_N[DEPTH]], BF16, name="mup", tag="mup")
        mup6a = pers.tile([128, 4, 512], BF16, name="mup6a", tag="mup6a")
        mup6b = pers.tile([128, 4, 512], BF16, name="mup6b", tag="mup6b")
        rmup = pers.tile([128, 4, NE], BF16, name="rmup", tag="rmup")
        U = pers.tile([128, 4, NL], BF16, name="U", tag="U")
        Urm = pers.tile([128, 4, NL], BF16, name="Urm", tag="Urm")

        def ar_ap(o):
            # ar columns [0, 496) live in ar04, [496, 1008) in ar5
            return (ar5, o - LVL_OFF[5]) if o >= LVL_OFF[5] else (ar04, o)

        def ml_ap(j):
            # leaf-edge m columns [0, 512) live in mup6a, [512, 1024) in mup6b
            return (mup6a, j) if j < 512 else (mup6b, j - 512)
        wb = {key: pers.tile([128, 4, 512], BF16, name=f"w_{key}", tag=f"w_{key}")
              for key, _, _ in WKEYS}
        ident_bf = pers.tile([128, 128], BF16, name="ident_bf", tag="ident_bf")

        ps_tags = ["pz", "ph", "pr", "pg"]
        rot = [0]

        # identity generated on-device (no DMA on the critical path)
        make_identity(nc, ident_bf[:])

        # ---- startup DMAs.  Whole tensors (2KB+ per-partition lines run at
        # ~300+ GB/s; strided slices run ~3x slower).  Wave 1 carries only
        # the leaf-critical bytes (ar5 / ur / ml) on the two HWDGE rings; the
        # bulk tensors sit behind gate DMAs reading compute-produced rmup so
        # they can't steal wave-1 HBM bandwidth.
        nc.sync.dma_start(ar5[:], ar5_d[:])
        nc.scalar.dma_start(wb["ur"][:], w_dram["ur"][:])
        nc.scalar.dma_start(mup6a[:], mla_d[:])
        nc.scalar.dma_start(mup6b[:], mlb_d[:])
        # sync ring: weights behind a gate so wave 1 is only the 2MB the
        # first leaf chunk needs (ar5 / ur / mla)
        nc.sync.dma_start(scr_d[:, 1:2], rmup[:, 0, E6:E6 + 1])
        nc.sync.dma_start(wb["wz2"][:], w_dram["wz2"][:])
        nc.sync.dma_start(wb["wh2"][:], w_dram["wh2"][:])
        # gpsimd ring: gate on leaf chunk-0 compute, then the bulk loads
        nc.gpsimd.dma_start(scr_d[:, 2:3], rmup[:, 0, E6:E6 + 1])
        nc.gpsimd.dma_start(ah[:], ah_d[:])
        nc.gpsimd.dma_start(ar04[:], ar04_d[:])

        def junk_mms(n, nn=128):
            # Real matmul burst into a scratch PSUM tile: the HAM clock-gate
            # only counts MATMUL activity (LDWEIGHTS doesn't un-throttle it),
            # and un-throttling needs a ~3.4us contiguous busy stretch.  The
            # results are never read.
            jp = ps.tile([128, 4, CH], F32, name="jp", tag=ps_tags[rot[0] % 4])
            rot[0] += 1
            rhs = ident_bf[:, :nn] if nn <= 128 else wb["ur"][:, 0, :nn]
            for i in range(n):
                nc.tensor.matmul(out=jp[:, 0, :nn], lhsT=ident_bf[:],
                                 rhs=rhs, start=(i == 0), stop=(i == n - 1))

        junk_mms(50)   # DMA-wait warm-up: ~5.4us of cold-clock MM activity

        def stage_b():
            # after leaf chunk 0: az behind a gate on the scalar ring; ag and
            # wg2 queue behind the gated weights on sync
            nc.scalar.dma_start(scr_d[:, 0:1], rmup[:, 0, E6:E6 + 1])
            nc.scalar.dma_start(az[:], az_d[:])
            nc.sync.dma_start(wb["wg2"][:], w_dram["wg2"][:])
            for a, b in [(0, NL), (NL, NN)]:
                nc.sync.dma_start(ag[:, :, a:b], ag_d[:, :, a:b])

        def ps_tile():
            t = ps.tile([128, 4, CH], F32, name="pp", tag=ps_tags[rot[0] % 4])
            rot[0] += 1
            return t

        def act2(out, in_, func):
            # split activation into two M-tile halves so downstream consumers
            # (per-k matmuls, DVE) start after half the work
            nc.scalar.activation(out[:, :2], in_[:, :2], func)
            nc.scalar.activation(out[:, 2:], in_[:, 2:], func)

        def tt2(eng, out, in0, in1, op):
            eng.tensor_tensor(out=out[:, :2], in0=in0[:, :2], in1=in1[:, :2], op=op)
            eng.tensor_tensor(out=out[:, 2:], in0=in0[:, 2:], in1=in1[:, 2:], op=op)

        def mm_pass(pt, nn, terms=(), inject=None, first=False, last=False):
            """Emit one ordered batch of matmuls accumulating into pt[:, :, :nn].

            PSUM ``has_written`` is per-element but ``start=True`` clears the
            whole 2KB bank, so a tile's matmuls carry exactly one start (first
            MM per bank, on the ``first=True`` batch) and one stop (last MM
            per bank, ``last=True`` batch).  terms: (weight_tile, rhs_fn(k))
            with [K, nn] APs.  inject: rhs_fn(m) returning a [128, nn] slice
            or a [128, 2, nn/2] outer-repeat AP, added via one identity-matmul
            per M-tile.
            """
            seq = []
            if inject is not None:
                for m in range(4):
                    seq.append((m, ident_bf[:], inject(m)))
            for wt, rhs_fn in terms:
                for m in range(4):
                    for k in range(4):
                        seq.append((m, wt[:KT[k], k, 128 * m:128 * (m + 1)], rhs_fn(k)))
            fb, lb = {}, {}
            for i, (m, _, _) in enumerate(seq):
                fb.setdefault(m // 2, i)
                lb[m // 2] = i
            for i, (m, lhsT, rhs) in enumerate(seq):
                out = pt[:, m, :nn]
                if len(rhs.shape) == 3:
                    out = out.rearrange("p (a b) -> p a b", a=2)
                nc.tensor.matmul(out=out, lhsT=lhsT, rhs=rhs,
                                 start=(first and fb[m // 2] == i),
                                 stop=(last and lb[m // 2] == i))

        def inj(t, o, n):         # plain inject of projection t cols [o, o+n)
            return lambda m: t[:, m, o:o + n]

        def inj_par(t, po, P, n0, nn):
            # inject indexed by parent: col j -> parent po + (j mod P)
            if nn <= P:
                pp = po + (n0 % P)
                return lambda m: t[:, m, pp:pp + nn]
            # chunk spans both halves (nn == 2P): outer-repeat the parent slice
            return lambda m: t[:, m:m + 1, po:po + P].broadcast_to((128, 2, P))

        # ============ leaf level (bottom-up l=6) ============
        # m_leaf comes pre-computed via DMA; device does the r-gate only.
        po6, P6 = LVL_OFF[DEPTH - 1], LVL_N[DEPTH] // 2
        for c in range(LVL_N[DEPTH] // CH):
            n0 = c * CH
            mlt, mo = ml_ap(n0)
            ms = mlt[:, :, mo:mo + CH]
            rms = rmup[:, :, E6 + n0:E6 + n0 + CH]

            pr = ps_tile()
            pp = n0 % P6
            mm_pass(pr, CH, inject=inj(ar5, pp, CH), first=True)
            mm_pass(pr, CH, [(wb["ur"], lambda k: mlt[:KT[k], k, mo:mo + CH])],
                    last=True)
            r = work.tile([128, 4, CH], BF16, name="r", tag="r")
            act2(r[:, :, :CH], pr[:, :, :CH], SIG)
            tt2(nc.vector, rms, r[:, :, :CH], ms, MUL)

            # pair-sums once the matching right-half chunk is done
            if c >= 2:
                j0 = (c - 2) * CH
                tt2(nc.gpsimd, U[:, :, po6 + j0:po6 + j0 + CH],
                    mup6a[:, :, j0:j0 + CH], mup6b[:, :, j0:j0 + CH], ADD)
                tt2(nc.vector, Urm[:, :, po6 + j0:po6 + j0 + CH],
                    rmup[:, :, E6 + j0:E6 + j0 + CH],
                    rmup[:, :, E6 + P6 + j0:E6 + P6 + j0 + CH], ADD)
            if c == 0:
                stage_b()

        # ================= phase 1: bottom-up (levels 5..1) =================
        for l in range(DEPTH - 1, 0, -1):
            L, o = LVL_N[l], LVL_OFF[l]
            e0, po, P = o - TPC, LVL_OFF[l - 1], LVL_N[l] // 2
            nch = max(1, L // CH)
            chn = min(CH, L)
            for c in range(nch):
                n0 = c * chn
                nn = chn
                ms = mup[:, :, e0 + n0:e0 + n0 + nn]
                rms = rmup[:, :, e0 + n0:e0 + n0 + nn]

                # recurrence-independent batches first
                pz = ps_tile()
                ph = ps_tile()
                pr = ps_tile()
                mm_pass(pz, nn, inject=inj(az, o + n0, nn), first=True)
                mm_pass(ph, nn, inject=inj(ah, o + n0, nn), first=True)
                mm_pass(pr, nn, inject=inj_par(ar04, po, P, n0, nn), first=True)

                z = work.tile([128, 4, CH], BF16, name="z", tag="z")
                mt = work.tile([128, 4, CH], BF16, name="mt", tag="mt")
                mm_pass(ph, nn, [(wb["wh2"], lambda k: Urm[:KT[k], k, o + n0:o + n0 + nn])],
                        last=True)
                act2(mt[:, :, :nn], ph[:, :, :nn], TANH)

                mm_pass(pz, nn, [(wb["wz2"], lambda k: U[:KT[k], k, o + n0:o + n0 + nn])],
                        last=True)
                act2(z[:, :, :nn], pz[:, :, :nn], SIG)

                s_ap = U[:, :, o + n0:o + n0 + nn]
                t1 = work.tile([128, 4, CH], BF16, name="t1", tag="t1")
                tt2(nc.vector, t1[:, :, :nn], mt[:, :, :nn], s_ap, SUB)
                t2 = work.tile([128, 4, CH], BF16, name="t2", tag="t2")
                tt2(nc.vector, t2[:, :, :nn], t1[:, :, :nn], z[:, :, :nn], MUL)
                tt2(nc.vector, ms, t2[:, :, :nn], s_ap, ADD)

                mm_pass(pr, nn, [(wb["ur"], lambda k: mup[:KT[k], k, e0 + n0:e0 + n0 + nn])],
                        last=True)
                r = work.tile([128, 4, CH], BF16, name="r", tag="r")
                act2(r[:, :, :nn], pr[:, :, :nn], SIG)
                tt2(nc.vector, rms, r[:, :, :nn], ms, MUL)

                # pair-sums to the parent level, chunk-wise as halves complete
                if c == nch - 1:
                    for j0 in range(0, P, CH):
                        pn = min(CH, P - j0)
                        tt2(nc.gpsimd, U[:, :, po + j0:po + j0 + pn],
                            mup[:, :, e0 + j0:e0 + j0 + pn],
                            mup[:, :, e0 + P + j0:e0 + P + j0 + pn], ADD)
                        tt2(nc.vector, Urm[:, :, po + j0:po + j0 + pn],
                            rmup[:, :, e0 + j0:e0 + j0 + pn],
                            rmup[:, :, e0 + P + j0:e0 + P + j0 + pn], ADD)
            if L <= 128:
                junk_mms(6)   # hold clock-gate density through the cascade

        # ================= roots output =================
        pg = ps_tile()
        mm_pass(pg, TPC, inject=inj(ag, 0, TPC), first=True)
        mm_pass(pg, TPC, [(wb["wg2"], lambda k: U[:KT[k], k, 0:TPC])], last=True)
        h0 = work.tile([128, 4, CH], BF16, name="h", tag="h")
        nc.scalar.activation(h0[:, :, :TPC], pg[:, :, :TPC], RELU)
        nc.sync.dma_start(out_d[0:384, 0:TPC].rearrange("(k p) c -> p k c", p=128),
                          h0[:, :3, :TPC])
        nc.sync.dma_start(out_d[384:450, 0:TPC], h0[:66, 3, :TPC])
        junk_mms(6)

        # ================= phase 2: top-down =================
        Tn = Trn = None
        for l in range(1, DEPTH + 1):
            L, o = LVL_N[l], LVL_OFF[l]
            e0, po, P = o - TPC, LVL_OFF[l - 1], LVL_N[l] // 2
            if l == 1:
                T_ap, Trm_ap = U[:, :, 0:TPC], Urm[:, :, 0:TPC]
            else:
                T_ap, Trm_ap = Tn[:, :, :P], Trn[:, :, :P]

            if l < DEPTH:
                Dm = dmp.tile([128, 4, LVL_N[DEPTH - 1]], BF16, name="Dm", tag="Dm")
                Drm = dmp.tile([128, 4, LVL_N[DEPTH - 1]], BF16, name="Drm", tag="Drm")
                Tn = dmp.tile([128, 4, 512], BF16, name="Tn", tag="Tn")
                Trn = dmp.tile([128, 4, 512], BF16, name="Trn", tag="Trn")

            nch = max(1, L // CH)
            chn = min(CH, L)
            for c in range(nch):
                n0 = c * chn
                nn = chn
                if l == DEPTH:
                    mlt, mo = ml_ap(n0)
                    mslice = mlt[:, :, mo:mo + nn]

                    def msl(a, b, mlt=mlt, mo=mo, n0=n0):
                        t, o2 = ml_ap(n0 + a)
                        return t[:, :, o2:o2 + b - a]
                else:
                    mslice = mup[:, :, e0 + n0:e0 + n0 + nn]

                    def msl(a, b, e0=e0, n0=n0):
                        return mup[:, :, e0 + n0 + a:e0 + n0 + b]
                rmslice = rmup[:, :, e0 + n0:e0 + n0 + nn]

                # recurrence-independent batches first
                pz = ps_tile()
                ph = ps_tile()
                pr = ps_tile() if l < DEPTH else None
                pg = ps_tile()
                mm_pass(pz, nn, inject=inj_par(az, po, P, n0, nn), first=True)
                mm_pass(ph, nn, inject=inj_par(ah, po, P, n0, nn), first=True)
                if pr is not None:
                    art, aro = ar_ap(o + n0)
                    mm_pass(pr, nn, inject=inj(art, aro, nn), first=True)
                mm_pass(pg, nn, inject=inj(ag, o + n0, nn), first=True)

                # s = T[parent] - m_up ; arm = Trm[parent] - rm_up
                # (contiguous thanks to the de-interleaved level layout)
                s = work.tile([128, 4, CH], BF16, name="s", tag="s")
                arm = work.tile([128, 4, CH], BF16, name="arm", tag="arm")
                if nn <= P:
                    pp = n0 % P
                    tt2(nc.vector, s[:, :, :nn], T_ap[:, :, pp:pp + nn], mslice, SUB)
                    tt2(nc.vector, arm[:, :, :nn], Trm_ap[:, :, pp:pp + nn],
                        rmslice, SUB)
                else:
                    tt2(nc.vector, s[:, :, :P], T_ap[:, :, 0:P], msl(0, P), SUB)
                    tt2(nc.vector, s[:, :, P:2 * P], T_ap[:, :, 0:P],
                        msl(P, 2 * P), SUB)
                    tt2(nc.vector, arm[:, :, :P], Trm_ap[:, :, 0:P],
                        rmup[:, :, e0 + n0:e0 + n0 + P], SUB)
                    tt2(nc.vector, arm[:, :, P:2 * P], Trm_ap[:, :, 0:P],
                        rmup[:, :, e0 + n0 + P:e0 + n0 + 2 * P], SUB)

                mm_pass(ph, nn, [(wb["wh2"], lambda k: arm[:KT[k], k, :nn])], last=True)
                mt = work.tile([128, 4, CH], BF16, name="mt", tag="mt")
                act2(mt[:, :, :nn], ph[:, :, :nn], TANH)

                mm_pass(pz, nn, [(wb["wz2"], lambda k: s[:KT[k], k, :nn])], last=True)
                z = work.tile([128, 4, CH], BF16, name="z", tag="z")
                act2(z[:, :, :nn], pz[:, :, :nn], SIG)

                if l < DEPTH:
                    dslice = Dm[:, :, n0:n0 + nn]
                else:
                    mb6 = work.tile([128, 4, CH], BF16, name="mb6", tag="nm")
                    dslice = mb6[:, :, :nn]
                t1 = work.tile([128, 4, CH], BF16, name="t1", tag="t1")
                tt2(nc.vector, t1[:, :, :nn], mt[:, :, :nn], s[:, :, :nn], SUB)
                t2 = work.tile([128, 4, CH], BF16, name="t2", tag="t2")
                tt2(nc.vector, t2[:, :, :nn], t1[:, :, :nn], z[:, :, :nn], MUL)
                tt2(nc.vector, dslice, t2[:, :, :nn], s[:, :, :nn], ADD)

                if l < DEPTH:
                    # r/rm feed the next level's arm; the last level has none
                    mm_pass(pr, nn, [(wb["ur"], lambda k: dslice[:KT[k], k, :])], last=True)
                    r = work.tile([128, 4, CH], BF16, name="r", tag="r")
                    act2(r[:, :, :nn], pr[:, :, :nn], SIG)
                    tt2(nc.vector, Drm[:, :, n0:n0 + nn], r[:, :, :nn], dslice, MUL)
                    # next level's Trm chunk
                    tt2(nc.vector, Trn[:, :, n0:n0 + nn],
                        Urm[:, :, o + n0:o + n0 + nn], Drm[:, :, n0:n0 + nn], ADD)

                # fused final output; node_m doubles as next level's T chunk
                if l == DEPTH:
                    nm_fn = lambda k: dslice[:KT[k], k, :]
                else:
                    tt2(nc.gpsimd, Tn[:, :, n0:n0 + nn],
                        U[:, :, o + n0:o + n0 + nn], dslice, ADD)
                    nm_fn = lambda k: Tn[:KT[k], k, n0:n0 + nn]
                mm_pass(pg, nn, [(wb["wg2"], nm_fn)], last=True)
                h = work.tile([128, 4, CH], BF16, name="h", tag="h")
                nc.scalar.activation(h[:, :, :nn], pg[:, :, :nn], RELU)
                nc.sync.dma_start(
                    out_d[0:384, o + n0:o + n0 + nn].rearrange("(k p) c -> p k c", p=128),
                    h[:, :3, :nn])
                nc.sync.dma_start(out_d[384:450, o + n0:o + n0 + nn], h[:66, 3, :nn])
            if L <= 128:
                junk_mms(10)   # hold clock-gate density through the cascade
            if l == 3:
                # escape burst: one contiguous ~3.4us cold-clock MM stretch
                # re-arms the HAM SHORT window before the dense big levels
                junk_mms(16, nn=CH)

    nc.compile()
    return nc


def _perm_for_core(c):
    """Node permutation: level-major, de-interleaved within each level.

    order(0) = [(t, 0) for t in trees]; order(l) = lefts(order(l-1)) then
    rights(order(l-1)), so children of the parent at in-level position j sit
    at positions j (left) and j + P (right).
    """
    perm = []
    order = [(t, 0) for t in range(TPC)]
    for l in range(DEPTH + 1):
        if l > 0:
            order = [(t, 2 * i) for (t, i) in order] + \
                    [(t, 2 * i + 1) for (t, i) in order]
        base_l = (1 << l) - 1
        for t, i in order:
            perm.append((TPC * c + t) * NPT + base_l + i)
    return np.asarray(perm, dtype=np.int64)


def _pack_kfmt(mat, ncols=None):
    """[N, 450] fp32 -> [128, 4, ncols] bf16 K-chunk layout (transposed)."""
    n = mat.shape[0] if ncols is None else ncols
    out = np.zeros((128, 4, n), dtype=BF16NP)
    for k in range(4):
        out[:KT[k], k, :] = mat[:n, KO[k]:KO[k] + KT[k]].T.astype(BF16NP)
    return out


def _pack_weight(W, ro):
    """W[ro:ro+450, :450] fp32 -> [128, 4, 512] bf16 lhsT (M zero-padded)."""
    out = np.zeros((128, 4, 512), dtype=BF16NP)
    for k in range(4):
        out[:KT[k], k, :H] = W[ro + KO[k]:ro + KO[k] + KT[k], :].astype(BF16NP)
    return out


def kernel(**inputs):
    wid = np.ascontiguousarray(np.asarray(inputs["wid"], dtype=np.int32))
    emb = np.ascontiguousarray(np.asarray(inputs["emb"], dtype=np.float32))
    ws = {nm: np.ascontiguousarray(np.asarray(inputs[nm], dtype=np.float32))
          for nm in ("Wz", "Wh", "Wr", "Ur", "Wg")}
    # biases are zero-filled by the reference generator; folding nonzero ones
    # into the per-vocab projections would be needed otherwise.
    for bn in ("bz", "bh", "bur", "bg"):
        bv = np.asarray(inputs[bn])
        assert not np.any(bv), f"nonzero bias {bn} unsupported by this kernel"

    if "nc" not in _CACHE:
        _CACHE["nc"] = _build_program()
        _CACHE["perms"] = [_perm_for_core(c) for c in range(NCORES)]
    nc = _CACHE["nc"]
    perms = _CACHE["perms"]

    # fold the embedding into the per-vocab projections once per vocab entry
    EZ = emb @ ws["Wz"][:H]
    EH = emb @ ws["Wh"][:H]
    ER = emb @ ws["Wr"]
    EG = emb @ ws["Wg"][:H]
    # leaf-level message is a pure per-vocab function: m = sigmoid(az)*tanh(ah)
    ML = (1.0 / (1.0 + np.exp(-EZ.astype(BF16NP).astype(np.float32)))
          * np.tanh(EH.astype(BF16NP).astype(np.float32)))
    wmaps = {key: _pack_weight(ws[srcnm], ro) for key, srcnm, ro in WKEYS}
    in_maps = []
    for c in range(NCORES):
        w = wid[perms[c]]
        wnl, wlf = w[:NL], w[NL:]
        ERp = ER[wnl]
        MLp = ML[wlf]
        m = {"azi": _pack_kfmt(EZ[wnl]), "ahi": _pack_kfmt(EH[wnl]),
             "agi": _pack_kfmt(EG[w]),
             "ar04i": _pack_kfmt(ERp[:LVL_OFF[5]]),
             "ar5i": _pack_kfmt(ERp[LVL_OFF[5]:]),
             "mlai": _pack_kfmt(MLp[:512]), "mlbi": _pack_kfmt(MLp[512:])}
        m.update(wmaps)
        in_maps.append(m)

    res = run_bass_kernel_spmd(nc, in_maps, core_ids=list(range(NCORES)))
    _CACHE["last_result"] = res

    out = np.empty((B * NPT, H), dtype=np.float32)
    for c in range(NCORES):
        out[perms[c]] = res.results[c]["hT"].T.astype(np.float32)
    return out


# revision 31
# speedup vs baseline: 1.0855x; 1.0155x over previous
"""DGL-JTNN encoder forest message passing on 8 Trainium2 NeuronCores.

Strategy: data-parallel over trees (16 complete binary trees per core, depth 6).
The forest built by the reference's ``_build_forest`` is deterministic complete
binary trees in BFS order, so the per-level segment-sums collapse into dense
ops.  v2 layout: within each level, nodes are DE-INTERLEAVED (all left children
first, then all right children, recursively), so

  * bottom-up pair-sum:  U = m[left half] + m[right half]       (contiguous TT)
  * top-down rep2:       s = T[parent slice] - m_up[half]        (contiguous TT)
  * rep2 injects become outer-repeat APs (one matmul, plain slices)

which keeps every DVE op in its fast 2x (bf16, step-1) mode.

All x-projections are folded into the embedding on the host (E@Wz1, E@Wh1,
E@Wr, E@Wg1 over the 780-entry vocab, then gathered per node); additionally the
whole leaf-level message m_leaf = sigmoid(az)*tanh(ah) is a per-vocab table
(leaves receive no messages), so it is gathered host-side and DMA'd straight
into the edge-state tile.  az/ah therefore only cover non-leaf nodes.

Feature-major tensors are [128, 4, N] SBUF tiles (feature 450 split
128/128/128/66); matmuls run bf16 with fp32 PSUM, PSUM tags rotated
round-robin; each PSUM bank carries exactly one start/stop so
recurrence-independent injects issue before recurrence-dependent matmuls.
Tiny levels use ar/ag injects (4 matmuls) instead of direct Wr/Wg1 passes
(16 matmuls) to cut the LDWEIGHTS-bound instruction count in the
latency-bound middle cascade.

DMA: inputs are spread need-ordered across the three DMA rings (sync HWDGE,
scalar HWDGE, gpsimd SWDGE); outputs (bf16) go on the sync ring.
"""

import sys

for _p in ("/opt/trn_rl_repo", "/root/.axon_site/_ro/trn_rl_repo"):
    if _p not in sys.path:
        sys.path.append(_p)

from contextlib import ExitStack

import numpy as np
import ml_dtypes

import concourse.bass as bass
import concourse.tile as tile
from concourse import bacc
from concourse import mybir
from concourse.bass_utils import run_bass_kernel_spmd
from concourse.masks import make_identity

F32 = mybir.dt.float32
BF16 = mybir.dt.bfloat16
SIG = mybir.ActivationFunctionType.Sigmoid
TANH = mybir.ActivationFunctionType.Tanh
RELU = mybir.ActivationFunctionType.Relu
ADD = mybir.AluOpType.add
SUB = mybir.AluOpType.subtract
MUL = mybir.AluOpType.mult

BF16NP = ml_dtypes.bfloat16

B, DEPTH, NPT, H, V = 128, 6, 127, 450, 780
NCORES = 8
TPC = B // NCORES                     # 16 trees per core
LVL_N = [TPC * (1 << l) for l in range(DEPTH + 1)]      # 16..1024
LVL_OFF = [0]
for n in LVL_N:
    LVL_OFF.append(LVL_OFF[-1] + n)
NN = LVL_OFF[-1]                      # 2032 nodes per core
NE = NN - TPC                         # 2016 up-edges per core
NL = LVL_OFF[DEPTH]                   # 1008 non-leaf cols
O6 = LVL_OFF[DEPTH]                   # leaf level node offset (1008)
E6 = O6 - TPC                         # leaf edge offset (992)
KT = [128, 128, 128, 66]              # feature K-chunk sizes (450 total)
KO = [0, 128, 256, 384]
CH = 256                              # N-chunk per pipeline step

# device weight blocks: (key, source tensor name, row offset)
WKEYS = [("wz2", "Wz", H), ("wh2", "Wh", H), ("ur", "Ur", 0), ("wg2", "Wg", H)]

_CACHE = {}


def _build_program():
    nc = bacc.Bacc("TRN2", target_bir_lowering=False, debug=False)

    az_d = nc.dram_tensor("azi", [128, 4, NL], BF16, kind="ExternalInput").ap()
    ah_d = nc.dram_tensor("ahi", [128, 4, NL], BF16, kind="ExternalInput").ap()
    ag_d = nc.dram_tensor("agi", [128, 4, NN], BF16, kind="ExternalInput").ap()
    # ar and the leaf-m table are split so every startup DMA is a whole
    # contiguous tensor (2KB+ per-partition lines; strided slices run ~3x
    # slower on the SDMA path)
    ar5_d = nc.dram_tensor("ar5i", [128, 4, LVL_N[5]], BF16, kind="ExternalInput").ap()
    ar04_d = nc.dram_tensor("ar04i", [128, 4, LVL_OFF[5]], BF16,
                            kind="ExternalInput").ap()
    mla_d = nc.dram_tensor("mlai", [128, 4, 512], BF16, kind="ExternalInput").ap()
    mlb_d = nc.dram_tensor("mlbi", [128, 4, 512], BF16, kind="ExternalInput").ap()
    scr_d = nc.dram_tensor("scr", [128, 8], BF16, kind="ExternalOutput").ap()
    w_dram = {key: nc.dram_tensor(key, [128, 4, 512], BF16, kind="ExternalInput").ap()
              for key, _, _ in WKEYS}
    out_d = nc.dram_tensor("hT", [H, NN], BF16, kind="ExternalOutput").ap()

    with tile.TileContext(nc) as tc, ExitStack() as ctx:
        pers = ctx.enter_context(tc.tile_pool(name="pers", bufs=1))
        work = ctx.enter_context(tc.tile_pool(name="work", bufs=2))
        dmp = ctx.enter_context(tc.tile_pool(name="dmp", bufs=2))
        ps = ctx.enter_context(tc.tile_pool(name="ps", bufs=1, space="PSUM"))

        # ---- persistent projection/state tiles ----
        az = pers.tile([128, 4, NL], BF16, name="az", tag="az")
        ah = pers.tile([128, 4, NL], BF16, name="ah", tag="ah")
        ag = pers.tile([128, 4, NN], BF16, name="ag", tag="ag")
        ar5 = pers.tile([128, 4, LVL_N[5]], BF16, name="ar5", tag="ar5")
        ar04 = pers.tile([128, 4, LVL_OFF[5]], BF16, name="ar04", tag="ar04")
        mup = pers.tile([128, 4, NE - LVL_N[DEPTH]], BF16, name="mup", tag="mup")
        mup6a = pers.tile([128, 4, 512], BF16, name="mup6a", tag="mup6a")
        mup6b = pers.tile([128, 4, 512], BF16, name="mup6b", tag="mup6b")
        rmup = pers.tile([128, 4, NE], BF16, name="rmup", tag="rmup")
        U = pers.tile([128, 4, NL], BF16, name="U", tag="U")
        Urm = pers.tile([128, 4, NL], BF16, name="Urm", tag="Urm")

        def ar_ap(o):
            # ar columns [0, 496) live in ar04, [496, 1008) in ar5
            return (ar5, o - LVL_OFF[5]) if o >= LVL_OFF[5] else (ar04, o)

        def ml_ap(j):
            # leaf-edge m columns [0, 512) live in mup6a, [512, 1024) in mup6b
            return (mup6a, j) if j < 512 else (mup6b, j - 512)
        wb = {key: pers.tile([128, 4, 512], BF16, name=f"w_{key}", tag=f"w_{key}")
              for key, _, _ in WKEYS}
        ident_bf = pers.tile([128, 128], BF16, name="ident_bf", tag="ident_bf")

        ps_tags = ["pz", "ph", "pr", "pg"]
        rot = [0]

        # identity generated on-device (no DMA on the critical path)
        make_identity(nc, ident_bf[:])

        # ---- startup DMAs.  Whole tensors (2KB+ per-partition lines run at
        # ~300+ GB/s; strided slices run ~3x slower).  Wave 1 carries only
        # the leaf-critical bytes (ar5 / ur / ml) on the two HWDGE rings; the
        # bulk tensors sit behind gate DMAs reading compute-produced rmup so
        # they can't steal wave-1 HBM bandwidth.
        # Wave 1 runs on the sync ring ALONE: concurrent rings round-robin at
        # packet granularity and collapse each other to ~1/3 line rate, so a
        # single ring in need-order is strictly faster.  The other two rings
        # open with gate DMAs (reads of compute-produced rmup) so they stay
        # silent until leaf compute is underway.
        nc.sync.dma_start(ar5[:], ar5_d[:])
        nc.sync.dma_start(wb["ur"][:], w_dram["ur"][:])
        nc.sync.dma_start(mup6a[:], mla_d[:])
        nc.sync.dma_start(mup6b[:], mlb_d[:])
        nc.sync.dma_start(wb["wz2"][:], w_dram["wz2"][:])
        nc.sync.dma_start(wb["wh2"][:], w_dram["wh2"][:])
        # gpsimd ring: gated on leaf chunk-1 compute, then bulk loads
        nc.gpsimd.dma_start(scr_d[:, 2:3], rmup[:, 0, E6 + 256:E6 + 257])
        nc.gpsimd.dma_start(ah[:], ah_d[:])
        nc.gpsimd.dma_start(ar04[:], ar04_d[:])

        def junk_mms(n, nn=128):
            # Real matmul burst into a scratch PSUM tile: the HAM clock-gate
            # only counts MATMUL activity (LDWEIGHTS doesn't un-throttle it),
            # and un-throttling needs a ~3.4us contiguous busy stretch.  The
            # results are never read.
            jp = ps.tile([128, 4, CH], F32, name="jp", tag=ps_tags[rot[0] % 4])
            rot[0] += 1
            rhs = ident_bf[:, :nn] if nn <= 128 else wb["ur"][:, 0, :nn]
            for i in range(n):
                nc.tensor.matmul(out=jp[:, 0, :nn], lhsT=ident_bf[:],
                                 rhs=rhs, start=(i == 0), stop=(i == n - 1))

        junk_mms(20)   # DMA-wait warm-up: cold-clock MM activity until ar5 lands

        def stage_b():
            # after leaf chunk 0: az behind a gate on the scalar ring; wg2/ag
            # queue on sync behind the wave-1 tensors
            nc.scalar.dma_start(scr_d[:, 0:1], rmup[:, 0, E6:E6 + 1])
            nc.scalar.dma_start(az[:], az_d[:])
            nc.sync.dma_start(wb["wg2"][:], w_dram["wg2"][:])
            for a, b in [(0, NL), (NL, NN)]:
                nc.sync.dma_start(ag[:, :, a:b], ag_d[:, :, a:b])

        def ps_tile():
            t = ps.tile([128, 4, CH], F32, name="pp", tag=ps_tags[rot[0] % 4])
            rot[0] += 1
            return t

        def act2(out, in_, func):
            # split activation into two M-tile halves so downstream consumers
            # (per-k matmuls, DVE) start after half the work
            nc.scalar.activation(out[:, :2], in_[:, :2], func)
            nc.scalar.activation(out[:, 2:], in_[:, 2:], func)

        def tt2(eng, out, in0, in1, op):
            eng.tensor_tensor(out=out[:, :2], in0=in0[:, :2], in1=in1[:, :2], op=op)
            eng.tensor_tensor(out=out[:, 2:], in0=in0[:, 2:], in1=in1[:, 2:], op=op)

        def mm_pass(pt, nn, terms=(), inject=None, first=False, last=False):
            """Emit one ordered batch of matmuls accumulating into pt[:, :, :nn].

            PSUM ``has_written`` is per-element but ``start=True`` clears the
            whole 2KB bank, so a tile's matmuls carry exactly one start (first
            MM per bank, on the ``first=True`` batch) and one stop (last MM
            per bank, ``last=True`` batch).  terms: (weight_tile, rhs_fn(k))
            with [K, nn] APs.  inject: rhs_fn(m) returning a [128, nn] slice
            or a [128, 2, nn/2] outer-repeat AP, added via one identity-matmul
            per M-tile.
            """
            seq = []
            if inject is not None:
                for m in range(4):
                    seq.append((m, ident_bf[:], inject(m)))
            for wt, rhs_fn in terms:
                for m in range(4):
                    for k in range(4):
                        seq.append((m, wt[:KT[k], k, 128 * m:128 * (m + 1)], rhs_fn(k)))
            fb, lb = {}, {}
            for i, (m, _, _) in enumerate(seq):
                fb.setdefault(m // 2, i)
                lb[m // 2] = i
            for i, (m, lhsT, rhs) in enumerate(seq):
                out = pt[:, m, :nn]
                if len(rhs.shape) == 3:
                    out = out.rearrange("p (a b) -> p a b", a=2)
                nc.tensor.matmul(out=out, lhsT=lhsT, rhs=rhs,
                                 start=(first and fb[m // 2] == i),
                                 stop=(last and lb[m // 2] == i))

        def inj(t, o, n):         # plain inject of projection t cols [o, o+n)
            return lambda m: t[:, m, o:o + n]

        def inj_par(t, po, P, n0, nn):
            # inject indexed by parent: col j -> parent po + (j mod P)
            if nn <= P:
                pp = po + (n0 % P)
                return lambda m: t[:, m, pp:pp + nn]
            # chunk spans both halves (nn == 2P): outer-repeat the parent slice
            return lambda m: t[:, m:m + 1, po:po + P].broadcast_to((128, 2, P))

        # ============ leaf level (bottom-up l=6) ============
        # m_leaf comes pre-computed via DMA; device does the r-gate only.
        po6, P6 = LVL_OFF[DEPTH - 1], LVL_N[DEPTH] // 2
        for c in range(LVL_N[DEPTH] // CH):
            n0 = c * CH
            mlt, mo = ml_ap(n0)
            ms = mlt[:, :, mo:mo + CH]
            rms = rmup[:, :, E6 + n0:E6 + n0 + CH]

            pr = ps_tile()
            pp = n0 % P6
            mm_pass(pr, CH, inject=inj(ar5, pp, CH), first=True)
            if c == 0:
                junk_mms(12)   # fill the inject->Ur DMA-wait, hold the clock
            mm_pass(pr, CH, [(wb["ur"], lambda k: mlt[:KT[k], k, mo:mo + CH])],
                    last=True)
            r = work.tile([128, 4, CH], BF16, name="r", tag="r")
            act2(r[:, :, :CH], pr[:, :, :CH], SIG)
            tt2(nc.vector, rms, r[:, :, :CH], ms, MUL)

            # pair-sums once the matching right-half chunk is done
            if c >= 2:
                j0 = (c - 2) * CH
                tt2(nc.gpsimd, U[:, :, po6 + j0:po6 + j0 + CH],
                    mup6a[:, :, j0:j0 + CH], mup6b[:, :, j0:j0 + CH], ADD)
                tt2(nc.vector, Urm[:, :, po6 + j0:po6 + j0 + CH],
                    rmup[:, :, E6 + j0:E6 + j0 + CH],
                    rmup[:, :, E6 + P6 + j0:E6 + P6 + j0 + CH], ADD)
            if c == 0:
                stage_b()

        # ================= phase 1: bottom-up (levels 5..1) =================
        for l in range(DEPTH - 1, 0, -1):
            L, o = LVL_N[l], LVL_OFF[l]
            e0, po, P = o - TPC, LVL_OFF[l - 1], LVL_N[l] // 2
            nch = max(1, L // CH)
            chn = min(CH, L)
            for c in range(nch):
                n0 = c * chn
                nn = chn
                ms = mup[:, :, e0 + n0:e0 + n0 + nn]
                rms = rmup[:, :, e0 + n0:e0 + n0 + nn]

                # recurrence-independent batches first
                pz = ps_tile()
                ph = ps_tile()
                pr = ps_tile()
                mm_pass(pz, nn, inject=inj(az, o + n0, nn), first=True)
                mm_pass(ph, nn, inject=inj(ah, o + n0, nn), first=True)
                mm_pass(pr, nn, inject=inj_par(ar04, po, P, n0, nn), first=True)

                z = work.tile([128, 4, CH], BF16, name="z", tag="z")
                mt = work.tile([128, 4, CH], BF16, name="mt", tag="mt")
                mm_pass(ph, nn, [(wb["wh2"], lambda k: Urm[:KT[k], k, o + n0:o + n0 + nn])],
                        last=True)
                act2(mt[:, :, :nn], ph[:, :, :nn], TANH)

                mm_pass(pz, nn, [(wb["wz2"], lambda k: U[:KT[k], k, o + n0:o + n0 + nn])],
                        last=True)
                act2(z[:, :, :nn], pz[:, :, :nn], SIG)

                s_ap = U[:, :, o + n0:o + n0 + nn]
                t1 = work.tile([128, 4, CH], BF16, name="t1", tag="t1")
                tt2(nc.vector, t1[:, :, :nn], mt[:, :, :nn], s_ap, SUB)
                t2 = work.tile([128, 4, CH], BF16, name="t2", tag="t2")
                tt2(nc.vector, t2[:, :, :nn], t1[:, :, :nn], z[:, :, :nn], MUL)
                tt2(nc.vector, ms, t2[:, :, :nn], s_ap, ADD)

                mm_pass(pr, nn, [(wb["ur"], lambda k: mup[:KT[k], k, e0 + n0:e0 + n0 + nn])],
                        last=True)
                r = work.tile([128, 4, CH], BF16, name="r", tag="r")
                act2(r[:, :, :nn], pr[:, :, :nn], SIG)
                tt2(nc.vector, rms, r[:, :, :nn], ms, MUL)

                # pair-sums to the parent level, chunk-wise as halves complete
                if c == nch - 1:
                    for j0 in range(0, P, CH):
                        pn = min(CH, P - j0)
                        tt2(nc.gpsimd, U[:, :, po + j0:po + j0 + pn],
                            mup[:, :, e0 + j0:e0 + j0 + pn],
                            mup[:, :, e0 + P + j0:e0 + P + j0 + pn], ADD)
                        tt2(nc.vector, Urm[:, :, po + j0:po + j0 + pn],
                            rmup[:, :, e0 + j0:e0 + j0 + pn],
                            rmup[:, :, e0 + P + j0:e0 + P + j0 + pn], ADD)
            if L <= 128:
                junk_mms(6)   # hold clock-gate density through the cascade

        # ================= roots output =================
        pg = ps_tile()
        mm_pass(pg, TPC, inject=inj(ag, 0, TPC), first=True)
        mm_pass(pg, TPC, [(wb["wg2"], lambda k: U[:KT[k], k, 0:TPC])], last=True)
        h0 = work.tile([128, 4, CH], BF16, name="h", tag="h")
        nc.scalar.activation(h0[:, :, :TPC], pg[:, :, :TPC], RELU)
        nc.sync.dma_start(out_d[0:384, 0:TPC].rearrange("(k p) c -> p k c", p=128),
                          h0[:, :3, :TPC])
        nc.sync.dma_start(out_d[384:450, 0:TPC], h0[:66, 3, :TPC])
        junk_mms(6)

        # ================= phase 2: top-down =================
        Tn = Trn = None
        for l in range(1, DEPTH + 1):
            L, o = LVL_N[l], LVL_OFF[l]
            e0, po, P = o - TPC, LVL_OFF[l - 1], LVL_N[l] // 2
            if l == 1:
                T_ap, Trm_ap = U[:, :, 0:TPC], Urm[:, :, 0:TPC]
            else:
                T_ap, Trm_ap = Tn[:, :, :P], Trn[:, :, :P]

            if l < DEPTH:
                Dm = dmp.tile([128, 4, LVL_N[DEPTH - 1]], BF16, name="Dm", tag="Dm")
                Drm = dmp.tile([128, 4, LVL_N[DEPTH - 1]], BF16, name="Drm", tag="Drm")
                Tn = dmp.tile([128, 4, 512], BF16, name="Tn", tag="Tn")
                Trn = dmp.tile([128, 4, 512], BF16, name="Trn", tag="Trn")

            nch = max(1, L // CH)
            chn = min(CH, L)
            for c in range(nch):
                n0 = c * chn
                nn = chn
                if l == DEPTH:
                    mlt, mo = ml_ap(n0)
                    mslice = mlt[:, :, mo:mo + nn]

                    def msl(a, b, mlt=mlt, mo=mo, n0=n0):
                        t, o2 = ml_ap(n0 + a)
                        return t[:, :, o2:o2 + b - a]
                else:
                    mslice = mup[:, :, e0 + n0:e0 + n0 + nn]

                    def msl(a, b, e0=e0, n0=n0):
                        return mup[:, :, e0 + n0 + a:e0 + n0 + b]
                rmslice = rmup[:, :, e0 + n0:e0 + n0 + nn]

                # recurrence-independent batches first
                pz = ps_tile()
                ph = ps_tile()
                pr = ps_tile() if l < DEPTH else None
                pg = ps_tile()
                mm_pass(pz, nn, inject=inj_par(az, po, P, n0, nn), first=True)
                mm_pass(ph, nn, inject=inj_par(ah, po, P, n0, nn), first=True)
                if pr is not None:
                    art, aro = ar_ap(o + n0)
                    mm_pass(pr, nn, inject=inj(art, aro, nn), first=True)
                mm_pass(pg, nn, inject=inj(ag, o + n0, nn), first=True)

                # s = T[parent] - m_up ; arm = Trm[parent] - rm_up
                # (contiguous thanks to the de-interleaved level layout)
                s = work.tile([128, 4, CH], BF16, name="s", tag="s")
                arm = work.tile([128, 4, CH], BF16, name="arm", tag="arm")
                if nn <= P:
                    pp = n0 % P
                    tt2(nc.vector, s[:, :, :nn], T_ap[:, :, pp:pp + nn], mslice, SUB)
                    tt2(nc.vector, arm[:, :, :nn], Trm_ap[:, :, pp:pp + nn],
                        rmslice, SUB)
                else:
                    tt2(nc.vector, s[:, :, :P], T_ap[:, :, 0:P], msl(0, P), SUB)
                    tt2(nc.vector, s[:, :, P:2 * P], T_ap[:, :, 0:P],
                        msl(P, 2 * P), SUB)
                    tt2(nc.vector, arm[:, :, :P], Trm_ap[:, :, 0:P],
                        rmup[:, :, e0 + n0:e0 + n0 + P], SUB)
                    tt2(nc.vector, arm[:, :, P:2 * P], Trm_ap[:, :, 0:P],
                        rmup[:, :, e0 + n0 + P:e0 + n0 + 2 * P], SUB)

                mm_pass(ph, nn, [(wb["wh2"], lambda k: arm[:KT[k], k, :nn])], last=True)
                mt = work.tile([128, 4, CH], BF16, name="mt", tag="mt")
                act2(mt[:, :, :nn], ph[:, :, :nn], TANH)

                mm_pass(pz, nn, [(wb["wz2"], lambda k: s[:KT[k], k, :nn])], last=True)
                z = work.tile([128, 4, CH], BF16, name="z", tag="z")
                act2(z[:, :, :nn], pz[:, :, :nn], SIG)

                if l < DEPTH:
                    dslice = Dm[:, :, n0:n0 + nn]
                else:
                    mb6 = work.tile([128, 4, CH], BF16, name="mb6", tag="nm")
                    dslice = mb6[:, :, :nn]
                t1 = work.tile([128, 4, CH], BF16, name="t1", tag="t1")
                tt2(nc.vector, t1[:, :, :nn], mt[:, :, :nn], s[:, :, :nn], SUB)
                t2 = work.tile([128, 4, CH], BF16, name="t2", tag="t2")
                tt2(nc.vector, t2[:, :, :nn], t1[:, :, :nn], z[:, :, :nn], MUL)
                tt2(nc.vector, dslice, t2[:, :, :nn], s[:, :, :nn], ADD)

                if l < DEPTH:
                    # r/rm feed the next level's arm; the last level has none
                    mm_pass(pr, nn, [(wb["ur"], lambda k: dslice[:KT[k], k, :])], last=True)
                    r = work.tile([128, 4, CH], BF16, name="r", tag="r")
                    act2(r[:, :, :nn], pr[:, :, :nn], SIG)
                    tt2(nc.vector, Drm[:, :, n0:n0 + nn], r[:, :, :nn], dslice, MUL)
                    # next level's Trm chunk
                    tt2(nc.vector, Trn[:, :, n0:n0 + nn],
                        Urm[:, :, o + n0:o + n0 + nn], Drm[:, :, n0:n0 + nn], ADD)

                # fused final output; node_m doubles as next level's T chunk
                if l == DEPTH:
                    nm_fn = lambda k: dslice[:KT[k], k, :]
                else:
                    tt2(nc.gpsimd, Tn[:, :, n0:n0 + nn],
                        U[:, :, o + n0:o + n0 + nn], dslice, ADD)
                    nm_fn = lambda k: Tn[:KT[k], k, n0:n0 + nn]
                mm_pass(pg, nn, [(wb["wg2"], nm_fn)], last=True)
                h = work.tile([128, 4, CH], BF16, name="h", tag="h")
                nc.scalar.activation(h[:, :, :nn], pg[:, :, :nn], RELU)
                nc.sync.dma_start(
                    out_d[0:384, o + n0:o + n0 + nn].rearrange("(k p) c -> p k c", p=128),
                    h[:, :3, :nn])
                nc.sync.dma_start(out_d[384:450, o + n0:o + n0 + nn], h[:66, 3, :nn])
            if L <= 128:
                junk_mms(10)   # hold clock-gate density through the cascade
            if l == 3:
                # escape burst: one contiguous ~3.4us cold-clock MM stretch
                # re-arms the HAM SHORT window before the dense big levels
                junk_mms(16, nn=CH)

    nc.compile()
    return nc


def _perm_for_core(c):
    """Node permutation: level-major, de-interleaved within each level.

    order(0) = [(t, 0) for t in trees]; order(l) = lefts(order(l-1)) then
    rights(order(l-1)), so children of the parent at in-level position j sit
    at positions j (left) and j + P (right).
    """
    perm = []
    order = [(t, 0) for t in range(TPC)]
    for l in range(DEPTH + 1):
        if l > 0:
            order = [(t, 2 * i) for (t, i) in order] + \
                    [(t, 2 * i + 1) for (t, i) in order]
        base_l = (1 << l) - 1
        for t, i in order:
            perm.append((TPC * c + t) * NPT + base_l + i)
    return np.asarray(perm, dtype=np.int64)


def _pack_kfmt(mat, ncols=None):
    """[N, 450] fp32 -> [128, 4, ncols] bf16 K-chunk layout (transposed)."""
    n = mat.shape[0] if ncols is None else ncols
    out = np.zeros((128, 4, n), dtype=BF16NP)
    for k in range(4):
        out[:KT[k], k, :] = mat[:n, KO[k]:KO[k] + KT[k]].T.astype(BF16NP)
    return out


def _pack_weight(W, ro):
    """W[ro:ro+450, :450] fp32 -> [128, 4, 512] bf16 lhsT (M zero-padded)."""
    out = np.zeros((128, 4, 512), dtype=BF16NP)
    for k in range(4):
        out[:KT[k], k, :H] = W[ro + KO[k]:ro + KO[k] + KT[k], :].astype(BF16NP)
    return out


def kernel(**inputs):
    wid = np.ascontiguousarray(np.asarray(inputs["wid"], dtype=np.int32))
    emb = np.ascontiguousarray(np.asarray(inputs["emb"], dtype=np.float32))
    ws = {nm: np.ascontiguousarray(np.asarray(inputs[nm], dtype=np.float32))
          for nm in ("Wz", "Wh", "Wr", "Ur", "Wg")}
    # biases are zero-filled by the reference generator; folding nonzero ones
    # into the per-vocab projections would be needed otherwise.
    for bn in ("bz", "bh", "bur", "bg"):
        bv = np.asarray(inputs[bn])
        assert not np.any(bv), f"nonzero bias {bn} unsupported by this kernel"

    if "nc" not in _CACHE:
        _CACHE["nc"] = _build_program()
        _CACHE["perms"] = [_perm_for_core(c) for c in range(NCORES)]
    nc = _CACHE["nc"]
    perms = _CACHE["perms"]

    # fold the embedding into the per-vocab projections once per vocab entry
    EZ = emb @ ws["Wz"][:H]
    EH = emb @ ws["Wh"][:H]
    ER = emb @ ws["Wr"]
    EG = emb @ ws["Wg"][:H]
    # leaf-level message is a pure per-vocab function: m = sigmoid(az)*tanh(ah)
    ML = (1.0 / (1.0 + np.exp(-EZ.astype(BF16NP).astype(np.float32)))
          * np.tanh(EH.astype(BF16NP).astype(np.float32)))
    wmaps = {key: _pack_weight(ws[srcnm], ro) for key, srcnm, ro in WKEYS}
    in_maps = []
    for c in range(NCORES):
        w = wid[perms[c]]
        wnl, wlf = w[:NL], w[NL:]
        ERp = ER[wnl]
        MLp = ML[wlf]
        m = {"azi": _pack_kfmt(EZ[wnl]), "ahi": _pack_kfmt(EH[wnl]),
             "agi": _pack_kfmt(EG[w]),
             "ar04i": _pack_kfmt(ERp[:LVL_OFF[5]]),
             "ar5i": _pack_kfmt(ERp[LVL_OFF[5]:]),
             "mlai": _pack_kfmt(MLp[:512]), "mlbi": _pack_kfmt(MLp[512:])}
        m.update(wmaps)
        in_maps.append(m)

    res = run_bass_kernel_spmd(nc, in_maps, core_ids=list(range(NCORES)))
    _CACHE["last_result"] = res

    out = np.empty((B * NPT, H), dtype=np.float32)
    for c in range(NCORES):
        out[perms[c]] = res.results[c]["hT"].T.astype(np.float32)
    return out
